# revision 54
# baseline (speedup 1.0000x reference)
"""Trainium2 Bass kernel for nn_DecoderWithoutAttention (image-caption LSTM decoder).

Strategy (zero-collective, batch-sharded over 8 NeuronCores):
  host:   mean-pool encoder feats, sort batch by caption length (desc),
          embedding gather, init h0/c0 + constant input projection (small GEMMs),
          reorder LSTM gate blocks [i,f,g,o] -> [i,f,o,g], build transposed
          device layouts. Each core gets 16 consecutive rows of the sorted batch.
  device: phase A: xproj[g,(t,b)] = W_ie^T-tiles @ emb^T (bf16 GEMM) + cproj
          phase B: 49 sequential LSTM steps, gates kept transposed
                   (gate dim on partitions, 16 batch cols on free dim);
                   masked state updates; masked h' history stored in bf16
          phase C: preds = Hmasked @ fc_W^T (bf16 GEMM, M=784, N=10000),
                   PSUM -> SBUF -> HBM as (16, 49, 10000) f32 per core
  host:   concatenate per-core outputs (already in sorted batch order).

Returns (predictions, caps_sorted, decode_lengths, sort_ind) like the reference.
"""

import numpy as np
import ml_dtypes

B, P_SP, D_ENC, L, V, E, H = 128, 8, 1280, 50, 10000, 512, 512
T = L - 1            # 49 decode steps
N_CORES = 8
BS = B // N_CORES    # 16 batch rows per core
G4 = 4 * H           # 2048 gate dims
NM = G4 // 128       # 16 gate m-tiles
KH = H // 128        # 4 contraction k-tiles
NV_CHUNK = 512       # vocab cols per matmul
NB_CH = 4            # vocab chunks per stream block
N_BLOCKS = (V + NV_CHUNK * NB_CH - 1) // (NV_CHUNK * NB_CH)  # 5
NMT = (T * BS + 127) // 128   # 7 row m-tiles in phase C (784 = 6*128 + 16)

BF16 = ml_dtypes.bfloat16

# gate blocks reordered from pytorch [i,f,g,o] to [i,f,o,g] so sigmoid covers
# a contiguous prefix; permutation applied to all gate-dim arrays on host
_GATE_PERM = np.concatenate([
    np.arange(0, H), np.arange(H, 2 * H), np.arange(3 * H, 4 * H),
    np.arange(2 * H, 3 * H)])


def _host_prep(encoder_out, encoded_captions, caption_lengths, emb_W, W_ih, W_hh,
               b_ih, b_hh, fc_W, fc_b, init_h_W, init_h_b, init_c_W, init_c_b):
    f32 = np.float32
    enc = encoder_out.reshape(B, -1, D_ENC).mean(axis=1, dtype=np.float64).astype(f32)
    lens = np.asarray(caption_lengths)[:, 0].astype(np.int64)
    sort_ind = np.argsort(-lens, kind="stable")
    lens_s = lens[sort_ind]
    enc_s = enc[sort_ind]
    caps_s = np.asarray(encoded_captions)[sort_ind]
    dec_len = lens_s - 1

    Wih = np.asarray(W_ih, f32)[_GATE_PERM]
    Whh = np.asarray(W_hh, f32)[_GATE_PERM]
    bias = (np.asarray(b_ih, f32) + np.asarray(b_hh, f32))[_GATE_PERM]
    W_ie = Wih[:, :E].copy()       # (2048, 512)
    W_id = Wih[:, E:]              # (2048, 1280)

    cproj = enc_s @ W_id.T.astype(f32) + bias          # (128, 2048)
    # single-tanh trick: device state stores H=2h, C=2c; one tanh(0.5*x)
    # yields sigmoid (via 2*sig-1) for i,f,o and exact tanh for g.
    #   ifo rows: pre = W_hh*h + xc = 0.5*W_hh*H + xc    -> W_hh_ifo *= 0.5
    #   g rows:   pre'= 2*(W_hh*h + xc) = W_hh*H + 2*xc  -> W_ie_g,cproj_g *= 2
    Whh_s = Whh.copy(); Whh_s[:3 * H] *= 0.5
    W_ie = W_ie.copy(); W_ie[3 * H:] *= 2.0
    cproj = cproj.copy(); cproj[:, 3 * H:] *= 2.0
    h0 = 2.0 * (enc_s @ np.asarray(init_h_W, f32).T + np.asarray(init_h_b, f32))
    c0 = 2.0 * (enc_s @ np.asarray(init_c_W, f32).T + np.asarray(init_c_b, f32))

    emb = np.asarray(emb_W, f32)[caps_s[:, :T].astype(np.int64)]  # (128, 49, 512)

    per_core = []
    for c in range(N_CORES):
        rc = slice(c * BS, (c + 1) * BS)
        # eT [512, 49*16], col = t*16+b
        eT = np.ascontiguousarray(
            emb[rc].transpose(2, 1, 0).reshape(H, T * BS)).astype(BF16)
        # cprojB [16, 2048] (batch on partitions) for the indicator-matmul add
        cprojB = np.ascontiguousarray(cproj[rc]).astype(BF16)
        h0T = np.ascontiguousarray(
            h0[rc].T.reshape(KH, 128, BS).transpose(1, 0, 2).reshape(128, KH * BS)
        ).astype(f32)
        c0T = np.ascontiguousarray(
            c0[rc].T.reshape(KH, 128, BS).transpose(1, 0, 2).reshape(128, KH * BS)
        ).astype(f32)
        # mask [49,16] -> [128, 49*64] with col = t*64 + j*16 + b
        mk = (np.arange(T)[:, None] < dec_len[rc][None, :]).astype(f32)  # (49,16)
        mk = np.broadcast_to(mk[:, None, :], (T, KH, BS)).reshape(1, T * KH * BS)
        maskT = np.ascontiguousarray(
            np.broadcast_to(mk, (128, T * KH * BS))).astype(BF16)
        maskU = np.ascontiguousarray(
            np.broadcast_to(mk, (128, T * KH * BS))).astype(np.uint8)
        per_core.append(dict(eT=eT, cprojB=cprojB, h0T=h0T, c0T=c0T,
                             maskT=maskT, maskU=maskU))

    indB = np.zeros((BS, 512), np.float32)
    indB[np.arange(512) % BS, np.arange(512)] = 1.0
    indB = indB.astype(BF16)
    shared = dict(
        indB=indB,
        wieT=np.ascontiguousarray(W_ie.T).astype(BF16),    # (512, 2048)
        whhT=np.ascontiguousarray(Whh_s.T).astype(BF16),   # (512, 2048)
        fcwT=np.ascontiguousarray(
            0.5 * np.asarray(fc_W, f32).T).astype(BF16),   # (512,10000)
    )
    meta = dict(caps_s=caps_s, dec_len=dec_len, sort_ind=sort_ind,
                fc_b=np.asarray(fc_b, f32))
    return shared, per_core, meta


def _finalize(per_core_out, meta, enc_cap_dtype, cap_len_dtype):
    preds = np.concatenate(per_core_out, axis=0)  # (128, 49, 10000) f32
    fc_b = meta["fc_b"]
    if np.any(fc_b):
        # reference adds fc_b before masking; our device path omits the
        # (all-zero in this problem) bias, so patch it in for active rows
        act = (np.arange(T)[None, :] < meta["dec_len"][:, None])
        preds = preds + act[:, :, None].astype(np.float32) * fc_b[None, None, :]
    caps = meta["caps_s"].astype(np.int32)
    dec_len = meta["dec_len"].astype(np.int32)
    sort_ind = meta["sort_ind"].astype(np.int32)
    return preds, caps, dec_len, sort_ind


# ---------------------------------------------------------------------------
# numpy emulation of the exact device pipeline (same layouts + bf16 rounding)
# ---------------------------------------------------------------------------

def _sigmoid(x):
    return 1.0 / (1.0 + np.exp(-x))


def _emulate_core(shared, ci, want_debug=False):
    f32 = np.float32
    eT = ci["eT"].astype(f32)          # (512, 784)
    wieT = shared["wieT"].astype(f32)  # (512, 2048) (g-rows pre-doubled)
    whhT = shared["whhT"].astype(f32)  # (512, 2048) (ifo-rows pre-halved)
    fcwT = shared["fcwT"].astype(f32)  # 0.5 * fc_W.T
    xproj = wieT.T @ eT                # (2048, 784)
    cproj_g = ci["cprojB"].astype(f32).T           # (2048, 16), bf16-rounded
    xc = (xproj.reshape(G4, T, BS) + cproj_g[:, None, :]).astype(BF16).astype(f32)

    Hst = ci["h0T"].reshape(128, KH, BS).transpose(1, 0, 2).reshape(H, BS)  # 2h
    Cst = ci["c0T"].reshape(128, KH, BS).transpose(1, 0, 2).reshape(H, BS)  # 2c
    Hmask = np.zeros((T, H, BS), f32)
    mk_full = ci["maskT"][0].astype(f32).reshape(T, KH * BS)
    for t in range(T):
        hb = Hst.astype(BF16).astype(f32)
        pre = whhT.T @ hb + xc[:, t, :]            # (2048, 16)
        gall = np.tanh(0.5 * pre)
        ip, fp, op_, tg = (gall[X * H:(X + 1) * H] for X in range(4))
        t1 = (fp + 1.0) * Cst
        t2 = (ip + 1.0) * tg
        cnX = 0.5 * t1 + t2                        # = 2*cn
        tc = np.tanh(0.5 * cnX)
        hnX = (op_ + 1.0) * tc                     # = 2*hn
        m = mk_full[t].reshape(KH, BS)[0][None, :]
        m = np.broadcast_to(m, (H, BS))
        Hmask[t] = (hnX * m).astype(BF16).astype(f32)
        Hst = Hst + m * (hnX - Hst)
        Cst = Cst + m * (cnX - Cst)
    Hm = Hmask.transpose(0, 2, 1).reshape(T * BS, H)   # rows t*16+b
    preds = (Hm.astype(BF16).astype(f32) @ fcwT).astype(BF16).astype(f32)
    out = preds.reshape(T, BS, V).transpose(1, 0, 2)  # (16, 49, 10000)
    if want_debug:
        xc_dev = xc.reshape(NM, 128, T, BS).transpose(1, 2, 0, 3).reshape(
            128, T * NM * BS)
        Hb_dev = Hmask.reshape(T, KH, 128, BS).transpose(2, 1, 0, 3).reshape(
            128, KH * T * BS).astype(BF16)
        h_dev = Hst.reshape(KH, 128, BS).transpose(1, 0, 2).reshape(128, KH * BS)
        c_dev = Cst.reshape(KH, 128, BS).transpose(1, 0, 2).reshape(128, KH * BS)
        dbg = dict(xc=xc_dev.astype(f32), Hb=Hb_dev, h=h_dev.astype(f32),
                   c=c_dev.astype(f32))
        return np.ascontiguousarray(out.astype(f32)), dbg
    return np.ascontiguousarray(out.astype(f32))


def kernel_emulate(**inputs):
    shared, per_core, meta = _host_prep(**inputs)
    outs = [_emulate_core(shared, per_core[c]) for c in range(N_CORES)]
    return _finalize(outs, meta, inputs["encoded_captions"].dtype,
                     inputs["caption_lengths"].dtype)


# ---------------------------------------------------------------------------
# device kernel
# ---------------------------------------------------------------------------

_BUILD_CACHE = {}


def _build_device_kernel(debug=False, phases="ABC"):
    key = ("nc", debug, phases)
    if key in _BUILD_CACHE:
        return _BUILD_CACHE[key]
    import concourse.bass as bass
    import concourse.tile as tile
    import concourse.mybir as mybir
    from concourse import bacc
    dt = mybir.dt
    AF = mybir.ActivationFunctionType

    nc = bacc.Bacc("TRN2", target_bir_lowering=False, debug=False)
    d_eT = nc.dram_tensor("eT", [H, T * BS], dt.bfloat16, kind="ExternalInput")
    d_wie = nc.dram_tensor("wieT", [H, G4], dt.bfloat16, kind="ExternalInput")
    d_whh = nc.dram_tensor("whhT", [H, G4], dt.bfloat16, kind="ExternalInput")
    d_fcw = nc.dram_tensor("fcwT", [H, V], dt.bfloat16, kind="ExternalInput")
    d_cproj = nc.dram_tensor("cprojB", [BS, G4], dt.bfloat16, kind="ExternalInput")
    d_indB = nc.dram_tensor("indB", [BS, 512], dt.bfloat16, kind="ExternalInput")
    d_h0 = nc.dram_tensor("h0T", [128, KH * BS], dt.float32, kind="ExternalInput")
    d_c0 = nc.dram_tensor("c0T", [128, KH * BS], dt.float32, kind="ExternalInput")
    d_mask = nc.dram_tensor("maskT", [128, T * KH * BS], dt.bfloat16, kind="ExternalInput")
    d_maskU = nc.dram_tensor("maskU", [128, T * KH * BS], dt.uint8, kind="ExternalInput")
    d_out = nc.dram_tensor("out", [BS, T, V], dt.bfloat16, kind="ExternalOutput")
    if debug:
        d_dbg_xc = nc.dram_tensor("dbg_xc", [128, T * NM * BS], dt.bfloat16,
                                  kind="ExternalOutput")
        d_dbg_Hb = nc.dram_tensor("dbg_Hb", [128, KH * T * BS], dt.bfloat16,
                                  kind="ExternalOutput")
        d_dbg_h = nc.dram_tensor("dbg_h", [128, KH * BS], dt.float32,
                                 kind="ExternalOutput")
        d_dbg_c = nc.dram_tensor("dbg_c", [128, KH * BS], dt.float32,
                                 kind="ExternalOutput")

    with tile.TileContext(nc) as tc:
        with tc.tile_pool(name="const", bufs=1) as cpool, \
             tc.tile_pool(name="big", bufs=1) as bigpool, \
             tc.tile_pool(name="tmp", bufs=3) as tpool, \
             tc.tile_pool(name="stage", bufs=3) as stpool:

            wie_sb = []
            whh_sb = []
            e_sb = []
            for k in range(KH):
                wt = cpool.tile([128, G4], dt.bfloat16, tag=f"wie{k}")
                nc.sync.dma_start(wt[:], d_wie.ap()[k * 128:(k + 1) * 128, :])
                wie_sb.append(wt)
            for k in range(KH):
                et = cpool.tile([128, T * BS], dt.bfloat16, tag=f"e{k}")
                nc.sync.dma_start(et[:], d_eT.ap()[k * 128:(k + 1) * 128, :])
                e_sb.append(et)
            for k in range(KH):
                wt = cpool.tile([128, G4], dt.bfloat16, tag=f"whh{k}",
                                name="whh_t")
                nc.sync.dma_start(wt[:], d_whh.ap()[k * 128:(k + 1) * 128, :])
                whh_sb.append(wt)
            cproj_sb = cpool.tile([BS, G4], dt.bfloat16, tag="cproj")
            nc.sync.dma_start(cproj_sb[:], d_cproj.ap()[:])
            indB_sb = cpool.tile([BS, 512], dt.bfloat16, tag="indB")
            nc.sync.dma_start(indB_sb[:], d_indB.ap()[:])
            mask_sb = cpool.tile([128, T * KH * BS], dt.bfloat16, tag="mask")
            nc.sync.dma_start(mask_sb[:], d_mask.ap()[:])
            masku_sb = cpool.tile([128, T * KH * BS], dt.uint8, tag="masku")
            nc.sync.dma_start(masku_sb[:], d_maskU.ap()[:])
            h0_sb = cpool.tile([128, KH * BS], dt.float32, tag="h0")
            nc.sync.dma_start(h0_sb[:], d_h0.ap()[:])
            c0_sb = cpool.tile([128, KH * BS], dt.float32, tag="c0")
            nc.sync.dma_start(c0_sb[:], d_c0.ap()[:])
            fcw_sb = []
            for k in range(KH):
                fw = cpool.tile([128, V], dt.bfloat16, tag=f"fcw{k}")
                nc.sync.dma_start(fw[:], d_fcw.ap()[k * 128:(k + 1) * 128, :])
                fcw_sb.append(fw)

            xc = bigpool.tile([128, T * NM * BS], dt.bfloat16, tag="xc")  # 3.2MB
            # masked h history, k-major: col = k*(T*BS) + t*BS + b so that
            # phase-C stationary tiles are contiguous slices
            Hb = bigpool.tile([128, KH * T * BS], dt.bfloat16, tag="Hb")  # 0.8MB
            Hb_w = Hb[:].rearrange("p (j t b) -> p j t b", j=KH, t=T, b=BS)

            out_ap = d_out.ap()
            xc_r = xc[:].rearrange("p (t m b) -> p t m b", t=T, m=NM, b=BS)

            # ---------------- phase A ----------------
            if "A" in phases:
                with tc.tile_pool(name="psA", bufs=2, space="PSUM") as psApool:
                    for chunk in (0, 1):
                        for m in range(NM):
                            n0, n1 = (0, 512) if chunk == 0 else (512, T * BS)
                            tlo, thi = (0, 32) if chunk == 0 else (32, T)
                            ps = psApool.tile([128, 512], dt.float32,
                                              tag="psA", name="psA")
                            for k in range(KH):
                                nc.tensor.matmul(
                                    ps[:, :n1 - n0],
                                    lhsT=wie_sb[k][:, m * 128:(m + 1) * 128],
                                    rhs=e_sb[k][:, n0:n1],
                                    start=(k == 0), stop=False)
                            # cproj broadcast over t via K=16 indicator matmul
                            nc.tensor.matmul(
                                ps[:, :n1 - n0],
                                lhsT=cproj_sb[:, m * 128:(m + 1) * 128],
                                rhs=indB_sb[:, :n1 - n0],
                                start=False, stop=True)
                            ps_r = ps[:, :n1 - n0].rearrange(
                                "p (t b) -> p t b", b=BS)
                            if m % 2 == 0:
                                nc.vector.tensor_copy(
                                    xc_r[:, tlo:thi, m, :], ps_r)
                            else:
                                nc.scalar.copy(xc_r[:, tlo:thi, m, :], ps_r)

            with tc.tile_pool(name="psum", bufs=1, space="PSUM") as pspool:

                fc_group = {"st": None, "g": -1, "m": -1, "filled": []}
                FCW = 2 * NV_CHUNK   # 1024-col fc jobs, one evacuation each

                def _fc_flush():
                    g = fc_group
                    if g["st"] is None or not g["filled"]:
                        return
                    m, grp = g["m"], g["g"]
                    t0 = 8 * m
                    ntr = min(8, T - t0)
                    rows = ntr * BS
                    nbase = grp * 2 * FCW
                    lo = min(g["filled"]) * FCW
                    hi = min(max(g["filled"]) * FCW + FCW, V - nbase)
                    dst = out_ap[:, t0:t0 + ntr,
                                 nbase + lo:nbase + hi].rearrange("b t w -> t b w")
                    nc.sync.dma_start(dst, g["st"][:rows, lo:hi])
                    g["st"] = None
                    g["filled"] = []

                def emit_fc(m, ci, eng):
                    """phase C job: row m-tile m, 1024-col vocab pair ci.
                    two jobs of the same (m, group) share a stage tile and one
                    output DMA (flushed when the group changes)."""
                    grp = ci // 2
                    if fc_group["m"] != m or fc_group["g"] != grp:
                        _fc_flush()
                        fc_group["m"], fc_group["g"] = m, grp
                    if fc_group["st"] is None:
                        fc_group["st"] = stpool.tile(
                            [128, 2 * FCW], dt.bfloat16, tag="stage",
                            name="fc_stage")
                    t0 = 8 * m
                    ntr = min(8, T - t0)
                    rows = ntr * BS
                    n0 = ci * FCW
                    w = min(FCW, V - n0)
                    ps = pspool.tile([128, FCW], dt.float32, tag="psC", bufs=3,
                                     name="psC")
                    for sub in range(2):
                        s0 = sub * NV_CHUNK
                        sw = min(NV_CHUNK, w - s0)
                        if sw <= 0:
                            break
                        for k in range(KH):
                            base = k * (T * BS) + t0 * BS
                            nc.tensor.matmul(
                                ps[:rows, s0:s0 + sw],
                                lhsT=Hb[:, base:base + rows],
                                rhs=fcw_sb[k][:, n0 + s0:n0 + s0 + sw],
                                start=(k == 0), stop=(k == KH - 1))
                    so = (ci % 2) * FCW
                    if eng == 0:
                        nc.vector.tensor_copy(fc_group["st"][:rows, so:so + w],
                                              ps[:rows, :w])
                    else:
                        nc.scalar.copy(fc_group["st"][:rows, so:so + w],
                                       ps[:rows, :w])
                    fc_group["filled"].append(ci % 2)

                # work queue drained inside the recurrence loop
                jobs = []
                if "C" in phases:
                    jobs += [("C", m, ci) for m in range(NMT - 1)
                             for ci in range((V + 2 * NV_CHUNK - 1) //
                                             (2 * NV_CHUNK))]
                jobs.reverse()  # pop() from the front of the logical order
                njob = 0

                def drain_jobs(t):
                    """emit interleavable fc jobs allowed at step t"""
                    nonlocal njob
                    budget = 1
                    while budget > 0 and jobs:
                        kind, m, x = jobs[-1]
                        if t < 8 * m + 8:
                            break  # Hb rows for this m-tile not ready yet
                        jobs.pop()
                        emit_fc(m, x, njob % 2)
                        njob += 1
                        budget -= 1

                # ---------------- phase B (+ interleaved A1/C jobs) --------
                # persistent state: c fp32, h bf16 (matmul input is
                # bf16-rounded every step anyway; frozen rows keep their bits)
                c_st = bigpool.tile([128, KH * BS], dt.float32, tag="c_st")
                nc.vector.tensor_copy(c_st[:], c0_sb[:])
                h_bf = bigpool.tile([128, KH * BS], dt.bfloat16, tag="hbf")
                nc.vector.tensor_copy(h_bf[:], h0_sb[:])
                nocell = "nocell" in phases

                def preload(t):
                    """allocate one psum tile for step t, preloaded with xc"""
                    ps = pspool.tile([128, 256], dt.float32, tag="ps_all",
                                     bufs=2, name="ps_all")
                    nc.vector.tensor_copy(ps[:], xc[:, t * 256:(t + 1) * 256])
                    return ps

                if "B" in phases:
                    ps_nxt = preload(0)
                for t in range(T if "B" in phases else 0):
                    mt = mask_sb[:, t * 64:(t + 1) * 64]
                    mtu = masku_sb[:, t * 64:(t + 1) * 64]
                    ps_all = ps_nxt
                    # matmuls accumulate onto the preloaded xc (walrus inserts
                    # the has_written fix)
                    for X in (0, 1, 2, 3):
                        for j in range(4):
                            m = 4 * X + j
                            for k in range(KH):
                                nc.tensor.matmul(
                                    ps_all[:, m * BS:(m + 1) * BS],
                                    lhsT=whh_sb[k][:, m * 128:(m + 1) * 128],
                                    rhs=h_bf[:, k * BS:(k + 1) * BS],
                                    start=False, stop=(X == 3 and j == 3 and
                                                       k == KH - 1),
                                    skip_group_check=True)
                    if nocell:
                        if t + 1 < T:
                            ps_nxt = preload(t + 1)
                        drain_jobs(t)
                        continue
                    # one activation: tanh(0.5*pre) = 2*sigmoid(pre)-1 for
                    # i,f,o (state is 2x-scaled) and exact tanh for g
                    gall = tpool.tile([128, 256], dt.float32, tag="gall")
                    nc.scalar.activation(gall[:], ps_all[:], AF.Tanh, scale=0.5)
                    ip = gall[:, 0:64]
                    fp = gall[:, 64:128]
                    op_ = gall[:, 128:192]
                    tg = gall[:, 192:256]
                    t1 = tpool.tile([128, 64], dt.float32, tag="t1")
                    nc.vector.scalar_tensor_tensor(
                        t1[:], fp, 1.0, c_st[:],
                        mybir.AluOpType.add, mybir.AluOpType.mult)
                    t2 = tpool.tile([128, 64], dt.float32, tag="t2")
                    nc.vector.scalar_tensor_tensor(
                        t2[:], ip, 1.0, tg,
                        mybir.AluOpType.add, mybir.AluOpType.mult)
                    cn = tpool.tile([128, 64], dt.float32, tag="cn")
                    nc.vector.scalar_tensor_tensor(
                        cn[:], t1[:], 0.5, t2[:],
                        mybir.AluOpType.mult, mybir.AluOpType.add)
                    tc_ = tpool.tile([128, 64], dt.float32, tag="tc")
                    nc.scalar.activation(tc_[:], cn[:], AF.Tanh, scale=0.5)
                    hn = tpool.tile([128, 64], dt.float32, tag="hn")
                    nc.vector.scalar_tensor_tensor(
                        hn[:], op_, 1.0, tc_[:],
                        mybir.AluOpType.add, mybir.AluOpType.mult)
                    # critical path: update h, then preload next step's psum
                    nc.vector.copy_predicated(h_bf[:], mtu, hn[:])
                    if t + 1 < T:
                        ps_nxt = preload(t + 1)
                    # off-path bookkeeping
                    nc.vector.copy_predicated(c_st[:], mtu, cn[:])
                    # masked bf16 history for fc (strided write into Hb)
                    nc.vector.tensor_mul(
                        Hb_w[:, :, t, :],
                        hn[:].rearrange("p (j b) -> p j b", j=KH, b=BS),
                        mt.rearrange("p (j b) -> p j b", j=KH, b=BS))
                    drain_jobs(t)

                if debug:
                    h_f32 = tpool.tile([128, KH * BS], dt.float32, tag="h_dbg")
                    nc.vector.tensor_copy(h_f32[:], h_bf[:])
                    nc.sync.dma_start(d_dbg_xc.ap()[:], xc[:])
                    nc.sync.dma_start(d_dbg_Hb.ap()[:], Hb[:])
                    nc.sync.dma_start(d_dbg_h.ap()[:], h_f32[:])
                    nc.sync.dma_start(d_dbg_c.ap()[:], c_st[:])

                # ---------------- phase C leftovers ----------------
                if "C" in phases:
                    rest = [("C", NMT - 1, ci)
                            for ci in range((V + 2 * NV_CHUNK - 1) //
                                            (2 * NV_CHUNK))]
                    for kind, m, ci in list(reversed(jobs)) + rest:
                        emit_fc(m, ci, njob % 2)
                        njob += 1
                    _fc_flush()
                    jobs.clear()
    nc.compile()
    _BUILD_CACHE[key] = nc
    return nc


def _run_device(shared, per_core, trace=False, trace_kwargs=None, debug=False):
    from concourse import bass_utils
    nc = _build_device_kernel(debug=debug)
    in_maps = []
    for c in range(N_CORES):
        m = dict(per_core[c])
        m.pop("_unused", None)
        m = {k: np.ascontiguousarray(v) for k, v in m.items()}
        m.update(shared)
        in_maps.append(m)
    res = bass_utils.run_bass_kernel_spmd(
        nc, in_maps, core_ids=list(range(N_CORES)), trace=trace,
        **(trace_kwargs or {}))
    outs = [np.asarray(res.results[c]["out"]).astype(np.float32)
            for c in range(N_CORES)]
    return outs, res


def _debug_compare(inputs, core=0):
    """Run debug build on HW, compare intermediates vs emulation for `core`."""
    shared, per_core, meta = _host_prep(**inputs)
    _, dbg_emu = _emulate_core(shared, per_core[core], want_debug=True)
    outs, res = _run_device(shared, per_core, debug=True)
    r = res.results[core]
    out_emu = _emulate_core(shared, per_core[core])
    for name, emu in [("xc", dbg_emu["xc"]), ("Hb", dbg_emu["Hb"]),
                      ("h", dbg_emu["h"]), ("c", dbg_emu["c"]),
                      ("out", out_emu)]:
        dev = np.asarray(r["out"] if name == "out" else r[f"dbg_{name}"],
                         np.float32)
        emu = np.asarray(emu, np.float32)
        d = np.abs(dev - emu)
        rel = np.linalg.norm(dev - emu) / max(np.linalg.norm(emu), 1e-30)
        print(f"  {name:4s}: rel={rel:.3e} maxabs={d.max():.3e} "
              f"dev[0,:4]={dev.ravel()[:4]} emu[0,:4]={emu.ravel()[:4]}")
    return res


def kernel(**inputs):
    shared, per_core, meta = _host_prep(**inputs)
    outs, _ = _run_device(shared, per_core)
    return _finalize(outs, meta, inputs["encoded_captions"].dtype,
                     inputs["caption_lengths"].dtype)


if __name__ == "__main__":
    # quick emulation self-check against a tiny local reproduction is in test.py
    pass
